# revision 59
# baseline (speedup 1.0000x reference)
"""AudioFinder Trainium2 kernel.

Data parallel over batch: 16 samples -> 8 cores x 2 samples.

Per-sample pipeline (all on one NeuronCore):
  1. 4-layer gated conv encoder on search (T=8192 -> 2040) and query
     (T=2048 -> 504).  Layer 0 (stride 2, k=4) is fed host-side im2col
     stacks in fp16: the 320-row contraction (80ch x 4 taps) is packed
     into 128/128/64-partition pieces, separately for even and odd
     output phases, chunk-major so each 512-col chunk is one contiguous
     slice -- 3 matmuls per gate half instead of 4, no deinterleave op,
     and the stride-2 output lands directly as even/odd streams.
     Layers 1-3 run f32r (full PE rate at free dim >= 256; weights are
     DMA'd raw and rounded on-chip via a small staging ring, a BIR
     verifier requirement).  Gated tanh*sigmoid on ACT, product on DVE
     (layer 3's on GpSimd), 1x1 + bias + residual fused in one DVE op.
     The final f0 1x1 is composed host-side with layer 3's 1x1 (two
     PSUM-accumulated matmuls) so x4 is never materialized.
  2. VQ: scores s[t,k] = enc_s[t]@emb[k] - |emb[k]|^2/2 via one f32r
     matmul per 128-t block (enc block stationary).  The nearest-codeword
     classifier contribution is recovered value-space: for j in {0,1}
       u_j[t] = max_k (s[t,k] + ew[k,j]/BIG),  m[t] = max_k s[t,k]
       => (u_j - m)*BIG = ew[argmax_k s, j]   (exact up to fp32 rounding)
     where ew = emb @ w_lin.T.  No argmax/gather instructions needed.
  3. v = enc_q @ w_lin.T + b_lin is tiled x4 (b_lin pad to 2040, -inf
     beyond) and brought into the same [128p, 16b] block layout as the
     VQ accumulators via a DRAM bounce, early and off the critical tail:
       z = (u_j - m)*BIG + v128,  out[s,j] = tanh(max z).
     The final 128-partition max runs on GpSimd (tensor_reduce axis=C).

Scheduling: the four encoder chains (2 samples x search/query) are
emitted as generators, round-robined at chunk granularity, each sample
wavefronted across layers so VQ work starts as early as its inputs
exist.  Tile tags are per-sample so the Tile scheduler can interleave
the dependency chains and keep the PE busy; input DMAs are chunked and
ordered by first use.
"""

import numpy as np

import concourse.bacc as bacc
import concourse.bass_isa as bass_isa
import concourse.mybir as mybir
import concourse.tile as tile
from concourse.bass_utils import run_bass_kernel_spmd

F32 = mybir.dt.float32
F32R = mybir.dt.float32r
F16 = mybir.dt.float16
AF = mybir.ActivationFunctionType
OP = mybir.AluOpType
AX = mybir.AxisListType

NCORES = 8
SPC = 2          # samples per core
C = 80
NK = 512         # codebook size
BIG = 1024.0
NEG = -1e30
CH = 512         # chunk (free-dim) size

M_F1 = 82  # f1 conv emits 80 real channels + const-1 channel + zero channel

# fp32 weight pack: [1x1_0 | L1 block | L2 block | L3 block | f0C | f0R | f1]
# where an Li block is [a j0..3 | g j0..3 | 1x1_i] (720 cols).
WPACK_COLS = 80 + 3 * 720 + 80 + 80 + M_F1  # 2482

def _w_off(kind, i=0, j=0):
    if kind == "a":
        return 80 + (i - 1) * 720 + j * C
    if kind == "g":
        return 80 + (i - 1) * 720 + 320 + j * C
    if kind == "1":
        return 0 if i == 0 else 80 + (i - 1) * 720 + 640
    if kind == "f0c":
        return 2240
    if kind == "f0r":
        return 2320
    if kind == "f1":
        return 2400
    raise KeyError(kind)


# bias pack columns: ba0..3, bg0..3, b10..3, bf0', bf1
def _b_off(kind, i=0):
    return {"a": i, "g": 4 + i, "1": 8 + i, "f0": 12, "f1": 13}[kind]


GEO_SEARCH = dict(E1=2048, O1=2047, T2=2046, T3=2043, T4=2040)
GEO_QUERY = dict(E1=512, O1=511, T2=510, T3=507, T4=504)

# search split into two independent column halves at t4=1024 (VQ-block
# aligned); the A half recomputes an 8-col layer-0 halo.  Plans are
# (c0_local, n); G = global column offset of the half.
HALF = {
    "A": dict(G=0, L0=[(0, 512), (512, 512), (1024, 8)],
              L1=(1030, [(0, 512), (512, 512), (1024, 6)]),
              L2=(1027, [(0, 512), (512, 512), (1024, 3)]),
              L3=[(0, 512), (512, 512)]),
    "B": dict(G=1024, L0=[(0, 512), (512, 512)],
              L1=(1022, [(0, 512), (512, 510)]),
              L2=(1019, [(0, 512), (512, 507)]),
              L3=[(0, 512), (512, 504)]),
}


def _build():
    nc = bacc.Bacc("TRN2", target_bir_lowering=False, debug=False,
                   num_devices=NCORES)
    d_sall = nc.dram_tensor("sall", [SPC, 128, 24 * CH], F16,
                            kind="ExternalInput")
    d_qall = nc.dram_tensor("qall", [SPC, 128, 6 * CH], F16,
                            kind="ExternalInput")
    d_wl0 = nc.dram_tensor("wl0", [128, 480], F16, kind="ExternalInput")
    d_wpk = nc.dram_tensor("wpk", [C, WPACK_COLS], F32, kind="ExternalInput")
    d_bpk = nc.dram_tensor("bpk", [M_F1, 17], F32, kind="ExternalInput")
    d_epk3 = nc.dram_tensor("epk3", [M_F1, 3 * NK], F32, kind="ExternalInput")
    d_out = nc.dram_tensor("out", [SPC, 2], F32, kind="ExternalOutput")
    d_zbuf = nc.dram_tensor("zbuf", [2 * SPC, 2048], F32)

    with tile.TileContext(nc) as tc:
        with (
            tc.tile_pool(name="sb", bufs=1) as sb,
            tc.tile_pool(name="ps", bufs=2, space="PSUM") as ps,
        ):
            # ---- static tables (DMA straight in; f32r is bit-identical).
            # DMA order follows first use: biases+L0 weights+query inputs,
            # then the rest.
            wl0 = sb.tile([128, 480], F16, tag="wl0")
            nc.sync.dma_start(wl0[:], d_wl0[:])
            bpk = sb.tile([M_F1, 17], F32, tag="bpk")
            nc.sync.dma_start(bpk[:], d_bpk[:])
            wpk = sb.tile([C, WPACK_COLS], F32R, tag="wpk")
            epk3 = sb.tile([M_F1, 3 * NK], F32R, tag="epk3")

            def stage_rounded(dst, d_src, rows, a):
                """DMA one fp32 chunk -> staging ring -> rounding copy into
                the f32r tile (BIR verifier: f32r matmul operands need a
                rounding producer)."""
                n = min(CH, dst.shape[1] - a)
                wstg = sb.tile([M_F1, CH], F32, tag="wstg", bufs=2,
                               name="wstg")
                nc.sync.dma_start(wstg[:rows, :n], d_src[:, a:a + n])
                nc.gpsimd.tensor_copy(dst[:, a:a + n], wstg[:rows, :n])

            def wsl(kind, i=0, j=0):
                off = _w_off(kind, i, j)
                n = M_F1 if kind == "f1" else C
                return wpk[:, off:off + n]

            def bap(kind, i=0):
                o = _b_off(kind, i)
                n = M_F1 if kind == "f1" else C
                return bpk[:n, o:o + 1]

            def zpad(t, a, b):
                # f32r memset is not a legal ISA op: write zeros via DVE
                # mult-by-0 of an already-loaded f32 tile (out dtype f32r)
                nc.vector.tensor_scalar(t[:, a:b], bpk[:C, 0:b - a], 0.0,
                                        None, op0=OP.mult)

            # ---------- per-sample persistent tiles ----------
            st = []
            for s in range(SPC):
                d = {}
                d["sall"] = sb.tile([128, 24 * CH], F16, tag=f"sall{s}",
                                    name=f"sall{s}")
                d["qall"] = sb.tile([128, 6 * CH], F16, tag=f"qall{s}",
                                    name=f"qall{s}")
                for h, (n1e, n1o, n2, n3) in (("A", (1032, 1031, 1030,
                                                     1027)),
                                              ("B", (1024, 1023, 1022,
                                                     1019))):
                    d[f"x1e{h}"] = sb.tile([C, n1e + 8], F32R,
                                           tag=f"x1e{h}{s}",
                                           name=f"x1e{h}{s}")
                    d[f"x1o{h}"] = sb.tile([C, n1o + 9], F32R,
                                           tag=f"x1o{h}{s}",
                                           name=f"x1o{h}{s}")
                    d[f"x2f{h}"] = sb.tile([C, n2 + 8], F32R,
                                           tag=f"x2f{h}{s}",
                                           name=f"x2f{h}{s}")
                    d[f"x3f{h}"] = sb.tile([C, n3 + 8], F32R,
                                           tag=f"x3f{h}{s}",
                                           name=f"x3f{h}{s}")
                    zpad(d[f"x1e{h}"], n1e, n1e + 8)
                    zpad(d[f"x1o{h}"], n1o, n1o + 9)
                    zpad(d[f"x2f{h}"], n2, n2 + 8)
                    zpad(d[f"x3f{h}"], n3, n3 + 8)
                d["qx1e"] = sb.tile([C, 512 + 8], F32R, tag="qx1e", bufs=2,
                                    name="qx1e")
                d["qx1o"] = sb.tile([C, 511 + 9], F32R, tag="qx1o", bufs=2,
                                    name="qx1o")
                d["qx2f"] = sb.tile([C, 510 + 8], F32R, tag="qx2f", bufs=2,
                                    name="qx2f")
                d["qx3f"] = sb.tile([C, 507 + 8], F32R, tag="qx3f", bufs=2,
                                    name="qx3f")
                zpad(d["qx1e"], 512, 520)
                zpad(d["qx1o"], 511, 520)
                zpad(d["qx2f"], 510, 518)
                zpad(d["qx3f"], 507, 515)
                d["encq"] = sb.tile([M_F1, 504], F32, tag="encq", bufs=2,
                                    name="encq")
                for nm in ("mt", "u0t", "u1t"):
                    d[nm] = sb.tile([128, 16], F32, tag=f"{nm}{s}",
                                    name=f"{nm}{s}")
                    nc.vector.memset(d[nm][:], NEG)
                d["v0"] = sb.tile([128, 16], F32, tag=f"v0_{s}",
                                  name=f"v0_{s}")
                d["v1"] = sb.tile([128, 16], F32, tag=f"v1_{s}",
                                  name=f"v1_{s}")
                st.append(d)
                nc.sync.dma_start(d["qall"][:, 0:3 * CH], d_qall[s, :, 0:3 * CH])

            # interleave weight staging chunks with per-chunk sall loads so
            # neither monopolizes the DMA path
            wsteps = ([(wpk, d_wpk, C, a) for a in range(0, WPACK_COLS, CH)]
                      + [(epk3, d_epk3, M_F1, a)
                         for a in range(0, 3 * NK, CH)])
            wi = 0

            def emit_wsteps(n):
                nonlocal wi
                for _ in range(n):
                    if wi < len(wsteps):
                        stage_rounded(*wsteps[wi])
                        wi += 1

            emit_wsteps(1)
            for s in range(SPC):
                nc.sync.dma_start(st[s]["sall"][:, 0:6 * CH],
                                  d_sall[s, :, 0:6 * CH])
                nc.sync.dma_start(st[s]["qall"][:, 3 * CH:],
                                  d_qall[s, :, 3 * CH:])
            for c in range(1, 4):
                for s in range(SPC):
                    nc.sync.dma_start(
                        st[s]["sall"][:, 6 * CH * c:6 * CH * (c + 1)],
                        d_sall[s, :, 6 * CH * c:6 * CH * (c + 1)])
                emit_wsteps(2)
            emit_wsteps(len(wsteps))

            zc = sb.tile([128, 4], F32, tag="zc")

            def gated(s, i, taps, psum_n, mm_pairs=None, mulp=False):
                """Wide conv + gated act.  taps = [(ap, off)] using wsl
                weights, or mm_pairs = per-half lists of (w_ap, x_ap).
                mulp routes the gated product to GpSimd instead of DVE.
                Returns the gated product (f32r [C, psum_n] SBUF tile)."""
                aps = ps.tile([C, psum_n], F32, tag="aps")
                gps = ps.tile([C, psum_n], F32, tag="gps")
                for hi, (half, pt) in enumerate((("a", aps), ("g", gps))):
                    if mm_pairs is not None:
                        pairs = mm_pairs[hi]
                    else:
                        pairs = [(wsl(half, i, j), src[:, off:off + psum_n])
                                 for j, (src, off) in enumerate(taps)]
                    npair = len(pairs)
                    for j, (w, x) in enumerate(pairs):
                        nc.tensor.matmul(pt[:], w, x, start=(j == 0),
                                         stop=(j == npair - 1))
                ta = sb.tile([C, psum_n], F32, tag="ta", bufs=3, name="ta")
                sg = sb.tile([C, psum_n], F32, tag="sg", bufs=3, name="sg")
                nc.scalar.activation(ta[:], aps[:], AF.Tanh, bias=bap("a", i))
                nc.scalar.activation(sg[:], gps[:], AF.Sigmoid, bias=bap("g", i))
                x2 = sb.tile([C, psum_n], F32R, tag="x2", bufs=4, name="x2")
                meng = nc.gpsimd if mulp else nc.vector
                meng.tensor_mul(x2[:], ta[:], sg[:])
                return x2

            def layer0(s, g, src, xe, xo, mulsel, plan=None, goff=0):
                """src = host-packed im2col stacks, chunk-major 512-col
                blocks: block (ci, k) at col (6*ci + k)*512, pieces k:
                e_A,e_B,e_C,o_A,o_B,o_C (C-pieces in rows 0:64).
                wl0: A-a,A-g,B-a,B-g,C-a,C-g.  plan gives (c0_local, n);
                goff = global offset of this half; sub-block halo entries
                (n < 512) address within their containing block."""
                E1, O1 = g["E1"], g["O1"]
                if plan is None:
                    plan = [(c0, CH) for c0 in range(0, E1, CH)]
                for c0, n in plan:
                    cg = goff + c0
                    blk = cg // 512
                    within = cg - 512 * blk
                    for half, (T_out, base, xdst) in enumerate(
                            ((E1 - goff, 0, xe), (O1 - goff, 3, xo))):
                        N = min(n, T_out - c0)
                        Nmm = N + (N & 1)
                        pairs = [
                            [(wl0[0:128, 160 * pc + 80 * hi:
                                  160 * pc + 80 * (hi + 1)][:64 if pc == 2
                                                            else 128, :],
                              src[:64 if pc == 2 else 128,
                                  (6 * blk + base + pc) * 512 + within:
                                  (6 * blk + base + pc) * 512 + within
                                  + Nmm])
                             for pc in range(3)]
                            for hi in range(2)]
                        x2 = gated(s, 0, None, Nmm, mm_pairs=pairs,
                                   mulp=mulsel(0))
                        xps = ps.tile([C, Nmm], F32, tag="xps")
                        nc.tensor.matmul(xps[:], wsl("1", 0), x2[:],
                                         start=True, stop=True)
                        if half == 0:
                            nc.vector.tensor_scalar(xdst[:, c0:c0 + N],
                                                    xps[:, :N], bap("1", 0),
                                                    None, op0=OP.add)
                        else:
                            nc.scalar.activation(xdst[:, c0:c0 + N],
                                                 xps[:, :N], AF.Identity,
                                                 bias=bap("1", 0))
                        yield

            def layer_mid(s, i, T_out, xin_taps, res, xout, mulsel,
                          plan=None):
                """stride-1/2 wide layer + 1x1 + bias + residual."""
                if plan is None:
                    plan = [(c0, min(CH, T_out - c0))
                            for c0 in range(0, T_out, CH)]
                for c0, N in plan:
                    Nmm = N + (N & 1)
                    x2 = gated(s, i, [(t, off + c0) for t, off in xin_taps],
                               Nmm, mulp=mulsel(i))
                    xps = ps.tile([C, Nmm], F32, tag="xps")
                    nc.tensor.matmul(xps[:], wsl("1", i), x2[:],
                                     start=True, stop=True)
                    rt, roff = res
                    nc.vector.scalar_tensor_tensor(
                        out=xout[:, c0:c0 + N], in0=xps[:, :N],
                        scalar=bap("1", i),
                        in1=rt[:, roff + c0:roff + c0 + N].bitcast(F32),
                        op0=OP.add, op1=OP.add)
                    yield

            def layer3_head(s, g, x3f, enc_full, vq_cb, mulsel,
                            plan=None):
                """layer 3 gated + (1x1_3∘f0 fused, residual tap accumulated
                in PSUM) + relu + f1.  Emits enc chunks; vq_cb(enc_t, c0, N,
                col offset of c0 inside enc_t); c0 is half-local."""
                T4 = g["T4"]
                if plan is None:
                    plan = [(c0, min(CH, T4 - c0))
                            for c0 in range(0, T4, CH)]
                for c0, N in plan:
                    Nmm = N + (N & 1)
                    x2 = gated(s, 3, [(x3f, c0), (x3f, c0 + 1), (x3f, c0 + 2),
                                      (x3f, c0 + 3)], Nmm, mulp=mulsel(3))
                    p0 = ps.tile([C, Nmm], F32, tag="xps")
                    nc.tensor.matmul(p0[:], wsl("f0c"), x2[:],
                                     start=True, stop=False)
                    nc.tensor.matmul(p0[:], wsl("f0r"),
                                     x3f[:, c0 + 3:c0 + 3 + Nmm],
                                     start=False, stop=True)
                    xf = sb.tile([C, Nmm], F32R, tag="xf", bufs=2, name="xf")
                    nc.scalar.activation(xf[:], p0[:], AF.Relu, bias=bap("f0"))
                    p1 = ps.tile([M_F1, Nmm], F32, tag="xps")
                    nc.tensor.matmul(p1[:], wsl("f1"), xf[:],
                                     start=True, stop=True)
                    if enc_full is not None:
                        nc.scalar.activation(enc_full[:, c0:c0 + N], p1[:, :N],
                                             AF.Identity, bias=bap("f1"))
                        if vq_cb is not None:
                            vq_cb(enc_full, c0, N, c0)
                        yield
                    else:
                        enc_t = sb.tile([M_F1, CH], F32R, tag="enc", bufs=2,
                                        name="enc")
                        nc.scalar.activation(enc_t[:, :N], p1[:, :N],
                                             AF.Identity, bias=bap("f1"))
                        if vq_cb is not None:
                            vq_cb(enc_t, c0, N, 0)
                        yield

            # ---------- emit: all four encoder chains, round-robined ------
            T4 = GEO_SEARCH["T4"]

            def make_vq(s, goff=0):
                d = st[s]

                def vq_cb(enc_t, c0, N, ebase):
                    cg = goff + c0
                    for b in range(cg // 128, (cg + N + 127) // 128):
                        t0 = 128 * b
                        P = min(128, T4 - t0)
                        for ti, tgt in ((0, d["mt"]), (1, d["u0t"]),
                                        (2, d["u1t"])):
                            sps = ps.tile([P, NK], F32, tag="vq")
                            nc.tensor.matmul(
                                sps[:], enc_t[:, t0 - cg + ebase:
                                              t0 - cg + ebase + P],
                                epk3[:, NK * ti:NK * (ti + 1)],
                                start=True, stop=True)
                            nc.vector.tensor_reduce(
                                tgt[:P, b:b + 1], sps[:], axis=AX.X,
                                op=OP.max)
                return vq_cb

            def enc_gen(s, is_query, h=None):
                """Yield once per emitted chunk so chains can be interleaved
                at emission time (slot-ring and priority order follow
                emission order).  For search, h selects the independent
                column half ("A"/"B")."""
                d = st[s]
                if is_query:
                    g, src_t = GEO_QUERY, d["qall"]
                    xe, xo = d["qx1e"], d["qx1o"]
                    x2f, x3f = d["qx2f"], d["qx3f"]
                    mulsel = lambda i=0: False
                else:
                    g, src_t = GEO_SEARCH, d["sall"]
                    xe, xo = d[f"x1e{h}"], d[f"x1o{h}"]
                    x2f, x3f = d[f"x2f{h}"], d[f"x3f{h}"]
                    mulsel = lambda i=0: i >= 3
                    hp = HALF[h]
                    G = hp["G"]
                if is_query:
                    l0 = layer0(s, g, src_t, xe, xo, mulsel)
                    l1 = layer_mid(s, 1, g["T2"],
                                   [(xe, 0), (xo, 0), (xe, 1), (xo, 1)],
                                   (xo, 1), x2f, mulsel)
                    l2 = layer_mid(s, 2, g["T3"],
                                   [(x2f, 0), (x2f, 1), (x2f, 2), (x2f, 3)],
                                   (x2f, 3), x3f, mulsel)
                    l3 = layer3_head(s, g, x3f, d["encq"], None, mulsel)
                    wave = [l0, l0, l1, l2, l3]
                else:
                    l0 = layer0(s, g, src_t, xe, xo, mulsel, plan=hp["L0"],
                                goff=G)
                    l1 = layer_mid(s, 1, hp["L1"][0],
                                   [(xe, 0), (xo, 0), (xe, 1), (xo, 1)],
                                   (xo, 1), x2f, mulsel, plan=hp["L1"][1])
                    l2 = layer_mid(s, 2, hp["L2"][0],
                                   [(x2f, 0), (x2f, 1), (x2f, 2), (x2f, 3)],
                                   (x2f, 3), x3f, mulsel, plan=hp["L2"][1])
                    l3 = layer3_head(s, g, x3f, None, make_vq(s, G), mulsel,
                                     plan=hp["L3"])
                    # wavefront within the half (2 chunks per layer + the
                    # layer-0 halo unit on A)
                    if h == "A":
                        wave = [l0, l0, l0, l0, l0, l0, l1, l1, l1, l2,
                                l2, l2, l3, l3]
                    else:
                        wave = [l0, l0, l0, l0, l1, l1, l2, l2, l3, l3]
                for gen_ in wave:
                    next(gen_)
                    yield
                if not is_query and h == "B":
                    # z = (u-m)*BIG + v per (sample, j): emitted right after
                    # this sample's VQ so it lands early in DVE's in-order
                    # queue instead of behind every other chain's work
                    for j in range(2):
                        dz = sb.tile([128, 16], F32, tag="dz", bufs=2,
                                     name="dz")
                        nc.vector.tensor_sub(dz[:], d[f"u{j}t"][:],
                                             d["mt"][:])
                        nc.vector.scalar_tensor_tensor(
                            out=dz[:], in0=dz[:], scalar=BIG,
                            in1=d[f"v{j}"][:], op0=OP.mult, op1=OP.add)
                        col = 2 * s + j
                        nc.vector.tensor_reduce(zc[:, col:col + 1], dz[:],
                                                axis=AX.X, op=OP.max)
                    yield
                if is_query:
                    # v = w_lin @ enc_q -> [2, 504]; tile x4 into zbuf
                    # rows via SWDGE (idle Pool queue), read back in the
                    # [128p, 16b] block layout
                    vps = ps.tile([2, 504], F32, tag="xps")
                    nc.tensor.matmul(vps[:], bpk[:C, 14:16], d["encq"][:C, :504],
                                     start=True, stop=True)
                    vsb = sb.tile([2, 536], F32, tag=f"vsb{s}",
                                  name=f"vsb{s}")
                    # fold b_lin into v so the final reduce needs no bias:
                    # real positions get v+b, zero-padded positions get b
                    for a in (504, 516):
                        nc.vector.tensor_scalar(vsb[:, a:a + 12],
                                                bpk[0:2, 0:12], 0.0,
                                                bpk[0:2, 16:17], op0=OP.mult,
                                                op1=OP.add)
                    nc.vector.memset(vsb[:, 528:536], NEG)
                    nc.scalar.activation(vsb[:, 0:504], vps[:], AF.Identity,
                                         bias=bpk[0:2, 16:17])
                    r0 = 2 * s
                    for j in range(2):
                        for k in range(4):
                            nc.sync.dma_start(
                                d_zbuf[r0 + j, 504 * k:504 * (k + 1)],
                                vsb[j:j + 1, 0:504])
                        nc.sync.dma_start(d_zbuf[r0 + j, 2016:2048],
                                          vsb[j:j + 1, 504:536])
                    for j in range(2):
                        nc.sync.dma_start(
                            d[f"v{j}"][:],
                            d_zbuf[r0 + j].rearrange("(b p) -> p b", p=128))
                    yield

            def half_rate(gen):
                # v isn't consumed until the z-phase: advance the query
                # chains every other round so search gets the early
                # PSUM/ACT bandwidth
                while True:
                    try:
                        next(gen)
                    except StopIteration:
                        return
                    yield
                    yield

            q0 = half_rate(enc_gen(0, True))
            q1 = half_rate(enc_gen(1, True))
            stage1 = [q0, q1, enc_gen(0, False, "A"), enc_gen(1, False, "A")]
            stage2 = [q0, q1, enc_gen(0, False, "B"), enc_gen(1, False, "B")]
            for gens in (stage1, stage2):
                gens = [g_ for g_ in gens if g_ is not None]
                while gens:
                    for gen in list(gens):
                        try:
                            next(gen)
                        except StopIteration:
                            gens.remove(gen)
                            if gen in stage2:
                                stage2.remove(gen)

            # ---------- tail: global partition max + tanh ----------
            # 128-partition max on GpSimd; b_lin already folded into v,
            # so just tanh and store
            zar = sb.tile([128, 4], F32, tag="zar")
            nc.gpsimd.partition_all_reduce(zar[:], zc[:], channels=128,
                                           reduce_op=bass_isa.ReduceOp.max)
            outv = sb.tile([1, 4], F32, tag="outv")
            nc.scalar.activation(outv[:], zar[0:1, :], AF.Tanh)
            nc.sync.dma_start(d_out[:], outv[:])

    nc.finalize()
    return nc


_NC_CACHE = None


def _get_nc():
    global _NC_CACHE
    if _NC_CACHE is None:
        _NC_CACHE = _build()
    return _NC_CACHE


def prep_inputs(search, query, w_wide, b_wide, w_1x1, b_1x1, w_f0, b_f0,
                w_f1, b_f1, embedding, w_lin, b_lin):
    """Host-side packing -> list of per-core input maps."""
    f = np.float32
    search = np.asarray(search, f)
    query = np.asarray(query, f)
    N = search.shape[0]

    def pack_l0(x):
        """[N, T, 80] -> chunk-major fp16 im2col stacks for the stride-2
        k=4 layer-0 conv: blocks (chunk c, piece k) of CH cols, pieces
        e_A, e_B, e_C, o_A, o_B, o_C (A/B/C = 128/128/64 contraction
        rows of the 320-row stack; C-pieces live in rows 0:64)."""
        S = [x[:, k::4, :].transpose(0, 2, 1) for k in range(4)]
        L = S[0].shape[2]           # stream length = E1
        se = np.concatenate(S, axis=1)              # [N, 320, L] even
        so = np.zeros_like(se)                      # odd: S2,S3,S0+1,S1+1
        so[:, 0:C, :L - 1] = S[2][:, :, :L - 1]
        so[:, C:2 * C, :L - 1] = S[3][:, :, :L - 1]
        so[:, 2 * C:3 * C, :L - 1] = S[0][:, :, 1:]
        so[:, 3 * C:4 * C, :L - 1] = S[1][:, :, 1:]
        nch = L // CH
        out = np.zeros((N, 128, 6 * nch * CH), np.float16)
        for c in range(nch):
            for k, (stk, r0) in enumerate(((se, 0), (se, 128), (se, 256),
                                           (so, 0), (so, 128), (so, 256))):
                rows = 64 if k % 3 == 2 else 128
                out[:, :rows, (6 * c + k) * CH:(6 * c + k) * CH + CH] = \
                    stk[:, r0:r0 + rows, c * CH:(c + 1) * CH]
        return out

    sall = pack_l0(search)
    qall = pack_l0(query)

    w_wide = np.asarray(w_wide, f)
    w_1x1 = np.asarray(w_1x1, f)
    w_f0 = np.asarray(w_f0, f)[:, :, 0]
    w_f1 = np.asarray(w_f1, f)[:, :, 0]
    # layer-0 stationaries: contraction rows r = 80*tap + ch, pieces
    # A/B/C = rows 0:128/128:256/256:320, halves a/g interleaved per piece
    wstack = np.zeros((320, 160), f)
    for tap in range(4):
        wstack[80 * tap:80 * (tap + 1), 0:80] = w_wide[0, :C, :, tap].T
        wstack[80 * tap:80 * (tap + 1), 80:160] = w_wide[0, C:, :, tap].T
    wl0 = np.zeros((128, 480), np.float16)
    wl0[:, 0:160] = wstack[0:128]
    wl0[:, 160:320] = wstack[128:256]
    wl0[0:64, 320:480] = wstack[256:320]

    cols = [w_1x1[0, :, :, 0].T]
    for i in range(1, 4):
        for j in range(4):
            cols.append(w_wide[i, :C, :, j].T)
        for j in range(4):
            cols.append(w_wide[i, C:, :, j].T)
        cols.append(w_1x1[i, :, :, 0].T)
    cols.append((w_f0 @ w_1x1[3, :, :, 0]).T)     # f0 composed with 1x1_3
    cols.append(w_f0.T)                           # f0 residual tap
    wf1 = np.zeros((C, M_F1), f)
    wf1[:, :C] = w_f1.T                           # cols 80/81 stay zero
    cols.append(wf1)
    wpk = np.ascontiguousarray(np.concatenate(cols, axis=1))
    assert wpk.shape == (C, WPACK_COLS)

    b_wide = np.asarray(b_wide, f)
    b_1x1 = np.asarray(b_1x1, f)
    bcols = [b_wide[i, :C] for i in range(4)]
    bcols += [b_wide[i, C:] for i in range(4)]
    bcols += [b_1x1[i] for i in range(4)]
    bcols += [np.asarray(b_f0, f) + w_f0 @ b_1x1[3], np.asarray(b_f1, f)]
    bpk = np.zeros((M_F1, 17), f)
    bpk[:C, :14] = np.stack(bcols, axis=1)
    bpk[C, _b_off("f1")] = 1.0   # f1 row 80 = 0*x + 1.0 -> const-1 channel
    bpk[:C, 14:16] = np.asarray(w_lin, f).T      # w_lin^T for the v matmul
    bpk[0:2, 16] = np.asarray(b_lin, f)          # b_lin folded into v

    emb = np.asarray(embedding, f)[0]            # (512, 80)
    e2 = (emb.astype(np.float64) ** 2).sum(1)
    ew = (emb.astype(np.float64) @ np.asarray(w_lin, f).T.astype(np.float64))
    epk3 = np.zeros((M_F1, 3 * NK), f)
    for ti in range(3):
        epk3[:C, NK * ti:NK * (ti + 1)] = emb.T
    epk3[C, 0:NK] = -0.5 * e2
    epk3[C, NK:2 * NK] = -0.5 * e2 + ew[:, 0] / BIG
    epk3[C, 2 * NK:3 * NK] = -0.5 * e2 + ew[:, 1] / BIG

    maps = []
    for c in range(NCORES):
        sl = slice(SPC * c, SPC * (c + 1))
        maps.append({
            "sall": np.ascontiguousarray(sall[sl]),
            "qall": np.ascontiguousarray(qall[sl]),
            "wl0": wl0, "wpk": wpk, "bpk": bpk, "epk3": epk3,
        })
    return maps


def kernel(**inputs):
    nc = _get_nc()
    maps = prep_inputs(**inputs)
    res = run_bass_kernel_spmd(nc, maps, core_ids=list(range(NCORES)))
    out = np.concatenate([r["out"] for r in res.results], axis=0)
    return out.astype(np.float32)


if __name__ == "__main__":
    import reference
    inputs = {k: np.asarray(v) for k, v in reference.setup_inputs().items()}
    got = kernel(**inputs)
    print(got)


# revision 69
# speedup vs baseline: 1.4867x; 1.4867x over previous
"""AudioFinder Trainium2 kernel.

Data parallel over batch: 16 samples -> 8 cores x 2 samples.

Per-sample pipeline (all on one NeuronCore):
  1. 4-layer gated conv encoder on search (T=8192 -> 2040) and query
     (T=2048 -> 504).  Layer 0 (stride 2, k=4) is fed host-side im2col
     stacks in fp16: the 320-row contraction (80ch x 4 taps) is packed
     into 128/128/64-partition pieces, separately for even and odd
     output phases, chunk-major so each 512-col chunk is one contiguous
     slice -- 3 matmuls per gate half instead of 4, no deinterleave op,
     and the stride-2 output lands directly as even/odd streams.
     Layers 1-3 run f32r (full PE rate at free dim >= 256; weights are
     DMA'd raw and rounded on-chip via a small staging ring, a BIR
     verifier requirement).  Gated tanh*sigmoid on ACT, product on DVE
     (layer 3's on GpSimd), 1x1 + bias + residual fused in one DVE op.
     The final f0 1x1 is composed host-side with layer 3's 1x1 (two
     PSUM-accumulated matmuls) so x4 is never materialized.
  2. VQ: scores s[t,k] = enc_s[t]@emb[k] - |emb[k]|^2/2 via one f32r
     matmul per 128-t block (enc block stationary).  The nearest-codeword
     classifier contribution is recovered value-space: for j in {0,1}
       u_j[t] = max_k (s[t,k] + ew[k,j]/BIG),  m[t] = max_k s[t,k]
       => (u_j - m)*BIG = ew[argmax_k s, j]   (exact up to fp32 rounding)
     where ew = emb @ w_lin.T.  No argmax/gather instructions needed.
  3. v = enc_q @ w_lin.T + b_lin is tiled x4 (b_lin pad to 2040, -inf
     beyond) and brought into the same [128p, 16b] block layout as the
     VQ accumulators via a DRAM bounce, early and off the critical tail:
       z = (u_j - m)*BIG + v128,  out[s,j] = tanh(max z).
     The final 128-partition max runs on GpSimd (tensor_reduce axis=C).

Scheduling: the four encoder chains (2 samples x search/query) are
emitted as generators, round-robined at chunk granularity, each sample
wavefronted across layers so VQ work starts as early as its inputs
exist.  Tile tags are per-sample so the Tile scheduler can interleave
the dependency chains and keep the PE busy; input DMAs are chunked and
ordered by first use.
"""

import numpy as np

import concourse.bacc as bacc
import concourse.bass_isa as bass_isa
import concourse.mybir as mybir
import concourse.tile as tile
from concourse.bass_utils import run_bass_kernel_spmd

F32 = mybir.dt.float32
F32R = mybir.dt.float32r
F16 = mybir.dt.float16
AF = mybir.ActivationFunctionType
OP = mybir.AluOpType
AX = mybir.AxisListType

NCORES = 8
SPC = 2          # samples per core
C = 80
NK = 512         # codebook size
BIG = 1024.0
NEG = -1e30
CH = 512         # chunk (free-dim) size

M_F1 = 82  # f1 conv emits 80 real channels + const-1 channel + zero channel

# fp32 weight pack: [1x1_0 | L1 block | L2 block | L3 block | f0C | f0R | f1]
# where an Li block is [a j0..3 | g j0..3 | 1x1_i] (720 cols).
WPACK_COLS = 80 + 3 * 720 + 80 + 80 + M_F1  # 2482

def _w_off(kind, i=0, j=0):
    if kind == "a":
        return 80 + (i - 1) * 720 + j * C
    if kind == "g":
        return 80 + (i - 1) * 720 + 320 + j * C
    if kind == "1":
        return 0 if i == 0 else 80 + (i - 1) * 720 + 640
    if kind == "f0c":
        return 2240
    if kind == "f0r":
        return 2320
    if kind == "f1":
        return 2400
    raise KeyError(kind)


# bias pack columns: ba0..3, bg0..3, b10..3, bf0', bf1
def _b_off(kind, i=0):
    return {"a": i, "g": 4 + i, "1": 8 + i, "f0": 12, "f1": 13}[kind]


GEO_SEARCH = dict(E1=2048, O1=2047, T2=2046, T3=2043, T4=2040)
GEO_QUERY = dict(E1=512, O1=511, T2=510, T3=507, T4=504)

# search split into two independent column halves at t4=1024 (VQ-block
# aligned); the A half recomputes an 8-col layer-0 halo.  Plans are
# (c0_local, n); G = global column offset of the half.
HALF = {
    "A": dict(G=0, L0=[(0, 512), (512, 512), (1024, 8)],
              L1=(1030, [(0, 512), (512, 512), (1024, 6)]),
              L2=(1027, [(0, 512), (512, 512), (1024, 3)]),
              L3=[(0, 512), (512, 512)]),
    "B": dict(G=1024, L0=[(0, 512), (512, 512)],
              L1=(1022, [(0, 512), (512, 510)]),
              L2=(1019, [(0, 512), (512, 507)]),
              L3=[(0, 512), (512, 504)]),
}


def _build():
    nc = bacc.Bacc("TRN2", target_bir_lowering=False, debug=False,
                   num_devices=NCORES)
    d_sall = nc.dram_tensor("sall", [SPC, 128, 24 * CH], F16,
                            kind="ExternalInput")
    d_qall = nc.dram_tensor("qall", [SPC, 128, 6 * CH], F16,
                            kind="ExternalInput")
    d_wl0 = nc.dram_tensor("wl0", [128, 480], F16, kind="ExternalInput")
    d_wpk = nc.dram_tensor("wpk", [C, WPACK_COLS], F32, kind="ExternalInput")
    d_bpk = nc.dram_tensor("bpk", [M_F1, 17], F32, kind="ExternalInput")
    d_epk3 = nc.dram_tensor("epk3", [M_F1, 3 * NK], F32, kind="ExternalInput")
    d_out = nc.dram_tensor("out", [SPC, 2], F32, kind="ExternalOutput")
    d_zbuf = nc.dram_tensor("zbuf", [2 * SPC, 2048], F32)

    with tile.TileContext(nc) as tc:
        with (
            tc.tile_pool(name="sb", bufs=1) as sb,
            tc.tile_pool(name="ps", bufs=2, space="PSUM") as ps,
        ):
            # ---- static tables (DMA straight in; f32r is bit-identical).
            # DMA order follows first use: biases+L0 weights+query inputs,
            # then the rest.
            wl0 = sb.tile([128, 480], F16, tag="wl0")
            nc.sync.dma_start(wl0[:], d_wl0[:])
            bpk = sb.tile([M_F1, 17], F32, tag="bpk")
            nc.sync.dma_start(bpk[:], d_bpk[:])
            wpk = sb.tile([C, WPACK_COLS], F32R, tag="wpk")
            epk3 = sb.tile([M_F1, 3 * NK], F32R, tag="epk3")

            def stage_rounded(dst, d_src, rows, a):
                """DMA one fp32 chunk -> staging ring -> rounding copy into
                the f32r tile (BIR verifier: f32r matmul operands need a
                rounding producer)."""
                n = min(CH, dst.shape[1] - a)
                wstg = sb.tile([M_F1, CH], F32, tag="wstg", bufs=2,
                               name="wstg")
                nc.sync.dma_start(wstg[:rows, :n], d_src[:, a:a + n])
                nc.gpsimd.tensor_copy(dst[:, a:a + n], wstg[:rows, :n])

            def wsl(kind, i=0, j=0):
                off = _w_off(kind, i, j)
                n = M_F1 if kind == "f1" else C
                return wpk[:, off:off + n]

            def bap(kind, i=0):
                o = _b_off(kind, i)
                n = M_F1 if kind == "f1" else C
                return bpk[:n, o:o + 1]

            def zpad(t, a, b):
                # f32r memset is not a legal ISA op: write zeros via DVE
                # mult-by-0 of an already-loaded f32 tile (out dtype f32r)
                nc.vector.tensor_scalar(t[:, a:b], bpk[:C, 0:b - a], 0.0,
                                        None, op0=OP.mult)

            # ---------- per-sample persistent tiles ----------
            st = []
            for s in range(SPC):
                d = {}
                d["sall"] = sb.tile([128, 24 * CH], F16, tag=f"sall{s}",
                                    name=f"sall{s}")
                d["qall"] = sb.tile([128, 6 * CH], F16, tag=f"qall{s}",
                                    name=f"qall{s}")
                for h, (n1e, n1o, n2, n3) in (("A", (1032, 1031, 1030,
                                                     1027)),
                                              ("B", (1024, 1023, 1022,
                                                     1019))):
                    d[f"x1e{h}"] = sb.tile([C, n1e + 8], F32R,
                                           tag=f"x1e{h}{s}",
                                           name=f"x1e{h}{s}")
                    d[f"x1o{h}"] = sb.tile([C, n1o + 9], F32R,
                                           tag=f"x1o{h}{s}",
                                           name=f"x1o{h}{s}")
                    d[f"x2f{h}"] = sb.tile([C, n2 + 8], F32R,
                                           tag=f"x2f{h}{s}",
                                           name=f"x2f{h}{s}")
                    d[f"x3f{h}"] = sb.tile([C, n3 + 8], F32R,
                                           tag=f"x3f{h}{s}",
                                           name=f"x3f{h}{s}")
                    zpad(d[f"x1e{h}"], n1e, n1e + 8)
                    zpad(d[f"x1o{h}"], n1o, n1o + 9)
                    zpad(d[f"x2f{h}"], n2, n2 + 8)
                    zpad(d[f"x3f{h}"], n3, n3 + 8)
                d["qx1e"] = sb.tile([C, 512 + 8], F32R, tag="qx1e", bufs=2,
                                    name="qx1e")
                d["qx1o"] = sb.tile([C, 511 + 9], F32R, tag="qx1o", bufs=2,
                                    name="qx1o")
                d["qx2f"] = sb.tile([C, 510 + 8], F32R, tag="qx2f", bufs=2,
                                    name="qx2f")
                d["qx3f"] = sb.tile([C, 507 + 8], F32R, tag="qx3f", bufs=2,
                                    name="qx3f")
                zpad(d["qx1e"], 512, 520)
                zpad(d["qx1o"], 511, 520)
                zpad(d["qx2f"], 510, 518)
                zpad(d["qx3f"], 507, 515)
                d["encq"] = sb.tile([M_F1, 504], F32, tag="encq", bufs=2,
                                    name="encq")
                for nm in ("mt", "u0t", "u1t"):
                    d[nm] = sb.tile([128, 16], F32, tag=f"{nm}{s}",
                                    name=f"{nm}{s}")
                    nc.vector.memset(d[nm][:], NEG)
                d["v0"] = sb.tile([128, 16], F32, tag=f"v0_{s}",
                                  name=f"v0_{s}")
                d["v1"] = sb.tile([128, 16], F32, tag=f"v1_{s}",
                                  name=f"v1_{s}")
                st.append(d)
                nc.sync.dma_start(d["qall"][:, 0:3 * CH], d_qall[s, :, 0:3 * CH])

            # interleave weight staging chunks with per-chunk sall loads so
            # neither monopolizes the DMA path
            wsteps = ([(wpk, d_wpk, C, a) for a in range(0, WPACK_COLS, CH)]
                      + [(epk3, d_epk3, M_F1, a)
                         for a in range(0, 3 * NK, CH)])
            wi = 0

            def emit_wsteps(n):
                nonlocal wi
                for _ in range(n):
                    if wi < len(wsteps):
                        stage_rounded(*wsteps[wi])
                        wi += 1

            emit_wsteps(1)
            for s in range(SPC):
                nc.sync.dma_start(st[s]["sall"][:, 0:6 * CH],
                                  d_sall[s, :, 0:6 * CH])
                nc.sync.dma_start(st[s]["qall"][:, 3 * CH:],
                                  d_qall[s, :, 3 * CH:])
            for c in range(1, 4):
                for s in range(SPC):
                    nc.sync.dma_start(
                        st[s]["sall"][:, 6 * CH * c:6 * CH * (c + 1)],
                        d_sall[s, :, 6 * CH * c:6 * CH * (c + 1)])
                emit_wsteps(2)
            emit_wsteps(len(wsteps))

            zc = sb.tile([128, 4], F32, tag="zc")

            def gated(s, i, taps, psum_n, mm_pairs=None, mulp=False):
                """Wide conv + gated act.  taps = [(ap, off)] using wsl
                weights, or mm_pairs = per-half lists of (w_ap, x_ap).
                mulp routes the gated product to GpSimd instead of DVE.
                Returns the gated product (f32r [C, psum_n] SBUF tile)."""
                aps = ps.tile([C, psum_n], F32, tag="aps")
                gps = ps.tile([C, psum_n], F32, tag="gps")
                for hi, (half, pt) in enumerate((("a", aps), ("g", gps))):
                    if mm_pairs is not None:
                        pairs = mm_pairs[hi]
                    else:
                        pairs = [(wsl(half, i, j), src[:, off:off + psum_n])
                                 for j, (src, off) in enumerate(taps)]
                    npair = len(pairs)
                    for j, (w, x) in enumerate(pairs):
                        nc.tensor.matmul(pt[:], w, x, start=(j == 0),
                                         stop=(j == npair - 1))
                ta = sb.tile([C, psum_n], F32, tag="ta", bufs=3, name="ta")
                sg = sb.tile([C, psum_n], F32, tag="sg", bufs=3, name="sg")
                nc.scalar.activation(ta[:], aps[:], AF.Tanh, bias=bap("a", i))
                nc.scalar.activation(sg[:], gps[:], AF.Sigmoid, bias=bap("g", i))
                x2 = sb.tile([C, psum_n], F32R, tag="x2", bufs=4, name="x2")
                meng = nc.gpsimd if mulp else nc.vector
                meng.tensor_mul(x2[:], ta[:], sg[:])
                return x2

            def layer0(s, g, src, xe, xo, mulsel, plan=None, goff=0):
                """src = host-packed im2col stacks, chunk-major 512-col
                blocks: block (ci, k) at col (6*ci + k)*512, pieces k:
                e_A,e_B,e_C,o_A,o_B,o_C (C-pieces in rows 0:64).
                wl0: A-a,A-g,B-a,B-g,C-a,C-g.  plan gives (c0_local, n);
                goff = global offset of this half; sub-block halo entries
                (n < 512) address within their containing block."""
                E1, O1 = g["E1"], g["O1"]
                if plan is None:
                    plan = [(c0, CH) for c0 in range(0, E1, CH)]
                for c0, n in plan:
                    cg = goff + c0
                    blk = cg // 512
                    within = cg - 512 * blk
                    for half, (T_out, base, xdst) in enumerate(
                            ((E1 - goff, 0, xe), (O1 - goff, 3, xo))):
                        N = min(n, T_out - c0)
                        Nmm = N + (N & 1)
                        pairs = [
                            [(wl0[0:128, 160 * pc + 80 * hi:
                                  160 * pc + 80 * (hi + 1)][:64 if pc == 2
                                                            else 128, :],
                              src[:64 if pc == 2 else 128,
                                  (6 * blk + base + pc) * 512 + within:
                                  (6 * blk + base + pc) * 512 + within
                                  + Nmm])
                             for pc in range(3)]
                            for hi in range(2)]
                        x2 = gated(s, 0, None, Nmm, mm_pairs=pairs,
                                   mulp=mulsel(0))
                        xps = ps.tile([C, Nmm], F32, tag="xps")
                        nc.tensor.matmul(xps[:], wsl("1", 0), x2[:],
                                         start=True, stop=True)
                        if half == 0:
                            nc.vector.tensor_scalar(xdst[:, c0:c0 + N],
                                                    xps[:, :N], bap("1", 0),
                                                    None, op0=OP.add)
                        else:
                            nc.scalar.activation(xdst[:, c0:c0 + N],
                                                 xps[:, :N], AF.Identity,
                                                 bias=bap("1", 0))
                        yield

            def layer_mid(s, i, T_out, xin_taps, res, xout, mulsel,
                          plan=None):
                """stride-1/2 wide layer + 1x1 + bias + residual."""
                if plan is None:
                    plan = [(c0, min(CH, T_out - c0))
                            for c0 in range(0, T_out, CH)]
                for c0, N in plan:
                    Nmm = N + (N & 1)
                    x2 = gated(s, i, [(t, off + c0) for t, off in xin_taps],
                               Nmm, mulp=mulsel(i))
                    xps = ps.tile([C, Nmm], F32, tag="xps")
                    nc.tensor.matmul(xps[:], wsl("1", i), x2[:],
                                     start=True, stop=True)
                    rt, roff = res
                    nc.vector.scalar_tensor_tensor(
                        out=xout[:, c0:c0 + N], in0=xps[:, :N],
                        scalar=bap("1", i),
                        in1=rt[:, roff + c0:roff + c0 + N].bitcast(F32),
                        op0=OP.add, op1=OP.add)
                    yield

            def layer3_head(s, g, x3f, enc_full, vq_cb, mulsel,
                            plan=None):
                """layer 3 gated + (1x1_3∘f0 fused, residual tap accumulated
                in PSUM) + relu + f1.  Emits enc chunks; vq_cb(enc_t, c0, N,
                col offset of c0 inside enc_t); c0 is half-local."""
                T4 = g["T4"]
                if plan is None:
                    plan = [(c0, min(CH, T4 - c0))
                            for c0 in range(0, T4, CH)]
                for c0, N in plan:
                    Nmm = N + (N & 1)
                    x2 = gated(s, 3, [(x3f, c0), (x3f, c0 + 1), (x3f, c0 + 2),
                                      (x3f, c0 + 3)], Nmm, mulp=mulsel(3))
                    p0 = ps.tile([C, Nmm], F32, tag="xps")
                    nc.tensor.matmul(p0[:], wsl("f0c"), x2[:],
                                     start=True, stop=False)
                    nc.tensor.matmul(p0[:], wsl("f0r"),
                                     x3f[:, c0 + 3:c0 + 3 + Nmm],
                                     start=False, stop=True)
                    xf = sb.tile([C, Nmm], F32R, tag="xf", bufs=2, name="xf")
                    nc.scalar.activation(xf[:], p0[:], AF.Relu, bias=bap("f0"))
                    p1 = ps.tile([M_F1, Nmm], F32, tag="xps")
                    nc.tensor.matmul(p1[:], wsl("f1"), xf[:],
                                     start=True, stop=True)
                    if enc_full is not None:
                        nc.scalar.activation(enc_full[:, c0:c0 + N], p1[:, :N],
                                             AF.Identity, bias=bap("f1"))
                        if vq_cb is not None:
                            vq_cb(enc_full, c0, N, c0)
                        yield
                    else:
                        enc_t = sb.tile([M_F1, CH], F32R, tag="enc", bufs=2,
                                        name="enc")
                        nc.scalar.activation(enc_t[:, :N], p1[:, :N],
                                             AF.Identity, bias=bap("f1"))
                        if vq_cb is not None:
                            vq_cb(enc_t, c0, N, 0)
                        yield

            # ---------- emit: all four encoder chains, round-robined ------
            T4 = GEO_SEARCH["T4"]

            def make_vq(s, goff=0):
                d = st[s]

                def vq_cb(enc_t, c0, N, ebase):
                    cg = goff + c0
                    for b in range(cg // 128, (cg + N + 127) // 128):
                        t0 = 128 * b
                        P = min(128, T4 - t0)
                        for ti, tgt in ((0, d["mt"]), (1, d["u0t"]),
                                        (2, d["u1t"])):
                            sps = ps.tile([P, NK], F32, tag="vq")
                            nc.tensor.matmul(
                                sps[:], enc_t[:, t0 - cg + ebase:
                                              t0 - cg + ebase + P],
                                epk3[:, NK * ti:NK * (ti + 1)],
                                start=True, stop=True)
                            nc.vector.tensor_reduce(
                                tgt[:P, b:b + 1], sps[:], axis=AX.X,
                                op=OP.max)
                return vq_cb

            def enc_gen(s, is_query, h=None):
                """Yield once per emitted chunk so chains can be interleaved
                at emission time (slot-ring and priority order follow
                emission order).  For search, h selects the independent
                column half ("A"/"B")."""
                d = st[s]
                if is_query:
                    g, src_t = GEO_QUERY, d["qall"]
                    xe, xo = d["qx1e"], d["qx1o"]
                    x2f, x3f = d["qx2f"], d["qx3f"]
                    mulsel = lambda i=0: False
                else:
                    g, src_t = GEO_SEARCH, d["sall"]
                    xe, xo = d[f"x1e{h}"], d[f"x1o{h}"]
                    x2f, x3f = d[f"x2f{h}"], d[f"x3f{h}"]
                    mulsel = lambda i=0: i >= 3
                    hp = HALF[h]
                    G = hp["G"]
                if is_query:
                    l0 = layer0(s, g, src_t, xe, xo, mulsel)
                    l1 = layer_mid(s, 1, g["T2"],
                                   [(xe, 0), (xo, 0), (xe, 1), (xo, 1)],
                                   (xo, 1), x2f, mulsel)
                    l2 = layer_mid(s, 2, g["T3"],
                                   [(x2f, 0), (x2f, 1), (x2f, 2), (x2f, 3)],
                                   (x2f, 3), x3f, mulsel)
                    l3 = layer3_head(s, g, x3f, d["encq"], None, mulsel)
                    wave = [l0, l0, l1, l2, l3]
                else:
                    l0 = layer0(s, g, src_t, xe, xo, mulsel, plan=hp["L0"],
                                goff=G)
                    l1 = layer_mid(s, 1, hp["L1"][0],
                                   [(xe, 0), (xo, 0), (xe, 1), (xo, 1)],
                                   (xo, 1), x2f, mulsel, plan=hp["L1"][1])
                    l2 = layer_mid(s, 2, hp["L2"][0],
                                   [(x2f, 0), (x2f, 1), (x2f, 2), (x2f, 3)],
                                   (x2f, 3), x3f, mulsel, plan=hp["L2"][1])
                    l3 = layer3_head(s, g, x3f, None, make_vq(s, G), mulsel,
                                     plan=hp["L3"])
                    # wavefront within the half: dive to deeper layers at
                    # the dependency frontier
                    if h == "A":
                        wave = [l0, l0, l0, l0, l0, l0, l1, l1, l1, l2,
                                l2, l2, l3, l3]
                    else:
                        wave = [l0, l0, l0, l0, l1, l1, l2, l2, l3, l3]
                for gen_ in wave:
                    next(gen_)
                    yield
                if not is_query and h == "B":
                    # z = (u-m)*BIG + v per (sample, j): emitted right after
                    # this sample's VQ so it lands early in DVE's in-order
                    # queue instead of behind every other chain's work
                    for j in range(2):
                        dz = sb.tile([128, 16], F32, tag="dz", bufs=2,
                                     name="dz")
                        nc.vector.tensor_sub(dz[:], d[f"u{j}t"][:],
                                             d["mt"][:])
                        nc.vector.scalar_tensor_tensor(
                            out=dz[:], in0=dz[:], scalar=BIG,
                            in1=d[f"v{j}"][:], op0=OP.mult, op1=OP.add)
                        col = 2 * s + j
                        nc.vector.tensor_reduce(zc[:, col:col + 1], dz[:],
                                                axis=AX.X, op=OP.max)
                    yield
                if is_query:
                    # v = w_lin @ enc_q -> [2, 504]; tile x4 into zbuf
                    # rows via SWDGE (idle Pool queue), read back in the
                    # [128p, 16b] block layout
                    vps = ps.tile([2, 504], F32, tag="xps")
                    nc.tensor.matmul(vps[:], bpk[:C, 14:16], d["encq"][:C, :504],
                                     start=True, stop=True)
                    vsb = sb.tile([2, 536], F32, tag=f"vsb{s}",
                                  name=f"vsb{s}")
                    # fold b_lin into v so the final reduce needs no bias:
                    # real positions get v+b, zero-padded positions get b
                    for a in (504, 516):
                        nc.vector.tensor_scalar(vsb[:, a:a + 12],
                                                bpk[0:2, 0:12], 0.0,
                                                bpk[0:2, 16:17], op0=OP.mult,
                                                op1=OP.add)
                    nc.vector.memset(vsb[:, 528:536], NEG)
                    nc.scalar.activation(vsb[:, 0:504], vps[:], AF.Identity,
                                         bias=bpk[0:2, 16:17])
                    r0 = 2 * s
                    for j in range(2):
                        for k in range(4):
                            nc.sync.dma_start(
                                d_zbuf[r0 + j, 504 * k:504 * (k + 1)],
                                vsb[j:j + 1, 0:504])
                        nc.sync.dma_start(d_zbuf[r0 + j, 2016:2048],
                                          vsb[j:j + 1, 504:536])
                    for j in range(2):
                        nc.sync.dma_start(
                            d[f"v{j}"][:],
                            d_zbuf[r0 + j].rearrange("(b p) -> p b", p=128))
                    yield

            def half_rate(gen):
                # v isn't consumed until the z-phase: advance the query
                # chains every other round so search gets the early
                # PSUM/ACT bandwidth
                while True:
                    try:
                        next(gen)
                    except StopIteration:
                        return
                    yield
                    yield

            # B-halves enter the rotation 10 rounds in: late enough not
            # to dilute A's chain overlap, early enough to soften the
            # phase boundary
            DK = 10

            def delayed(gen, k):
                for _ in range(k):
                    yield
                while True:
                    try:
                        next(gen)
                    except StopIteration:
                        return
                    yield

            q0 = enc_gen(0, True)
            q1 = enc_gen(1, True)
            gens = [q0, q1, enc_gen(0, False, "A"), enc_gen(1, False, "A"),
                    delayed(enc_gen(0, False, "B"), DK),
                    delayed(enc_gen(1, False, "B"), DK)]
            while gens:
                for gen in list(gens):
                    try:
                        next(gen)
                    except StopIteration:
                        gens.remove(gen)

            # ---------- tail: global partition max + tanh ----------
            # 128-partition max on GpSimd; b_lin already folded into v,
            # so just tanh and store
            zar = sb.tile([128, 4], F32, tag="zar")
            nc.gpsimd.partition_all_reduce(zar[:], zc[:], channels=128,
                                           reduce_op=bass_isa.ReduceOp.max)
            outv = sb.tile([1, 4], F32, tag="outv")
            nc.scalar.activation(outv[:], zar[0:1, :], AF.Tanh)
            nc.sync.dma_start(d_out[:], outv[:])

    nc.finalize()
    return nc


_NC_CACHE = None


def _get_nc():
    global _NC_CACHE
    if _NC_CACHE is None:
        _NC_CACHE = _build()
    return _NC_CACHE


def prep_inputs(search, query, w_wide, b_wide, w_1x1, b_1x1, w_f0, b_f0,
                w_f1, b_f1, embedding, w_lin, b_lin):
    """Host-side packing -> list of per-core input maps."""
    f = np.float32
    search = np.asarray(search, f)
    query = np.asarray(query, f)
    N = search.shape[0]

    def pack_l0(x):
        """[N, T, 80] -> chunk-major fp16 im2col stacks for the stride-2
        k=4 layer-0 conv: blocks (chunk c, piece k) of CH cols, pieces
        e_A, e_B, e_C, o_A, o_B, o_C (A/B/C = 128/128/64 contraction
        rows of the 320-row stack; C-pieces live in rows 0:64)."""
        S = [x[:, k::4, :].transpose(0, 2, 1) for k in range(4)]
        L = S[0].shape[2]           # stream length = E1
        se = np.concatenate(S, axis=1)              # [N, 320, L] even
        so = np.zeros_like(se)                      # odd: S2,S3,S0+1,S1+1
        so[:, 0:C, :L - 1] = S[2][:, :, :L - 1]
        so[:, C:2 * C, :L - 1] = S[3][:, :, :L - 1]
        so[:, 2 * C:3 * C, :L - 1] = S[0][:, :, 1:]
        so[:, 3 * C:4 * C, :L - 1] = S[1][:, :, 1:]
        nch = L // CH
        out = np.zeros((N, 128, 6 * nch * CH), np.float16)
        for c in range(nch):
            for k, (stk, r0) in enumerate(((se, 0), (se, 128), (se, 256),
                                           (so, 0), (so, 128), (so, 256))):
                rows = 64 if k % 3 == 2 else 128
                out[:, :rows, (6 * c + k) * CH:(6 * c + k) * CH + CH] = \
                    stk[:, r0:r0 + rows, c * CH:(c + 1) * CH]
        return out

    sall = pack_l0(search)
    qall = pack_l0(query)

    w_wide = np.asarray(w_wide, f)
    w_1x1 = np.asarray(w_1x1, f)
    w_f0 = np.asarray(w_f0, f)[:, :, 0]
    w_f1 = np.asarray(w_f1, f)[:, :, 0]
    # layer-0 stationaries: contraction rows r = 80*tap + ch, pieces
    # A/B/C = rows 0:128/128:256/256:320, halves a/g interleaved per piece
    wstack = np.zeros((320, 160), f)
    for tap in range(4):
        wstack[80 * tap:80 * (tap + 1), 0:80] = w_wide[0, :C, :, tap].T
        wstack[80 * tap:80 * (tap + 1), 80:160] = w_wide[0, C:, :, tap].T
    wl0 = np.zeros((128, 480), np.float16)
    wl0[:, 0:160] = wstack[0:128]
    wl0[:, 160:320] = wstack[128:256]
    wl0[0:64, 320:480] = wstack[256:320]

    cols = [w_1x1[0, :, :, 0].T]
    for i in range(1, 4):
        for j in range(4):
            cols.append(w_wide[i, :C, :, j].T)
        for j in range(4):
            cols.append(w_wide[i, C:, :, j].T)
        cols.append(w_1x1[i, :, :, 0].T)
    cols.append((w_f0 @ w_1x1[3, :, :, 0]).T)     # f0 composed with 1x1_3
    cols.append(w_f0.T)                           # f0 residual tap
    wf1 = np.zeros((C, M_F1), f)
    wf1[:, :C] = w_f1.T                           # cols 80/81 stay zero
    cols.append(wf1)
    wpk = np.ascontiguousarray(np.concatenate(cols, axis=1))
    assert wpk.shape == (C, WPACK_COLS)

    b_wide = np.asarray(b_wide, f)
    b_1x1 = np.asarray(b_1x1, f)
    bcols = [b_wide[i, :C] for i in range(4)]
    bcols += [b_wide[i, C:] for i in range(4)]
    bcols += [b_1x1[i] for i in range(4)]
    bcols += [np.asarray(b_f0, f) + w_f0 @ b_1x1[3], np.asarray(b_f1, f)]
    bpk = np.zeros((M_F1, 17), f)
    bpk[:C, :14] = np.stack(bcols, axis=1)
    bpk[C, _b_off("f1")] = 1.0   # f1 row 80 = 0*x + 1.0 -> const-1 channel
    bpk[:C, 14:16] = np.asarray(w_lin, f).T      # w_lin^T for the v matmul
    bpk[0:2, 16] = np.asarray(b_lin, f)          # b_lin folded into v

    emb = np.asarray(embedding, f)[0]            # (512, 80)
    e2 = (emb.astype(np.float64) ** 2).sum(1)
    ew = (emb.astype(np.float64) @ np.asarray(w_lin, f).T.astype(np.float64))
    epk3 = np.zeros((M_F1, 3 * NK), f)
    for ti in range(3):
        epk3[:C, NK * ti:NK * (ti + 1)] = emb.T
    epk3[C, 0:NK] = -0.5 * e2
    epk3[C, NK:2 * NK] = -0.5 * e2 + ew[:, 0] / BIG
    epk3[C, 2 * NK:3 * NK] = -0.5 * e2 + ew[:, 1] / BIG

    maps = []
    for c in range(NCORES):
        sl = slice(SPC * c, SPC * (c + 1))
        maps.append({
            "sall": np.ascontiguousarray(sall[sl]),
            "qall": np.ascontiguousarray(qall[sl]),
            "wl0": wl0, "wpk": wpk, "bpk": bpk, "epk3": epk3,
        })
    return maps


def kernel(**inputs):
    nc = _get_nc()
    maps = prep_inputs(**inputs)
    res = run_bass_kernel_spmd(nc, maps, core_ids=list(range(NCORES)))
    out = np.concatenate([r["out"] for r in res.results], axis=0)
    return out.astype(np.float32)


if __name__ == "__main__":
    import reference
    inputs = {k: np.asarray(v) for k, v in reference.setup_inputs().items()}
    got = kernel(**inputs)
    print(got)
